# revision 6
# baseline (speedup 1.0000x reference)
"""Bilinear decoder kernel for Trainium2 (8 NeuronCores, SPMD).

Computes score[b] = head[b]^T @ relation_matrices[relation_ids[b]] @ tail[b]
for b in [0, 4096).

Strategy:
  Host: group samples by relation id, chunk each group into tiles of <=128
  rows (so every tile uses exactly one relation matrix), pad the tile list
  so each of the 8 cores gets the same tile count (SPMD: one program).
  Device (per tile): indirect-DMA gather the tile's 256 relation-matrix rows
  (as 2x [128,256] SBUF tiles), two accumulated fp32 matmuls
  psum[128b,256j] += headT_chunk[128i,128b].T @ M_chunk[128i,256j], then a
  fused DVE multiply+reduce against the tail tile to get per-row scores.
  Host: scatter scores back through the sort permutation.
"""

import numpy as np

P = 128
DIM = 256
NCORES = 8

_prog_cache = {}

# test-harness knobs: set TRACE=True before calling kernel() to capture an
# NTFF profile; the BassKernelResults lands in LAST_RESULT.
TRACE = False
LAST_RESULT = None


def _build(n_tiles):
    import concourse.bass as bass
    import concourse.bacc as bacc
    import concourse.mybir as mybir
    import concourse.tile as tile

    f32 = mybir.dt.float32
    i32 = mybir.dt.int32

    # Bacc (not raw Bass): its compile() runs move_matmul_waits_to_ldweights +
    # generate_event_semaphores, required on TRN2 where an instruction can
    # carry at most one sync wait (raw Bass fails walrus codegen here).
    nc = bacc.Bacc("TRN2", target_bir_lowering=False)
    headT = nc.dram_tensor("headT", [n_tiles, 2, P, P], f32, kind="ExternalInput")
    tailt = nc.dram_tensor("tailt", [n_tiles, P, DIM], f32, kind="ExternalInput")
    gidxT = nc.dram_tensor("gidxT", [P, n_tiles * 2], i32, kind="ExternalInput")
    mrows = nc.dram_tensor("mrows", [30 * DIM, DIM], f32, kind="ExternalInput")
    out = nc.dram_tensor("out", [P, n_tiles], f32, kind="ExternalOutput")

    with tile.TileContext(nc) as tc:
        with (
            tc.tile_pool(name="const", bufs=1) as const_pool,
            tc.tile_pool(name="io", bufs=4) as io_pool,
            tc.tile_pool(name="mat", bufs=4) as m_pool,
            tc.tile_pool(name="psum", bufs=4, space="PSUM") as psum_pool,
        ):
            gidx_sb = const_pool.tile([P, n_tiles * 2], i32)
            nc.sync.dma_start(out=gidx_sb[:], in_=gidxT[:, :])
            out_sb = const_pool.tile([P, n_tiles], f32)

            for t in range(n_tiles):
                m0 = m_pool.tile([P, DIM], f32, tag="m")
                m1 = m_pool.tile([P, DIM], f32, tag="m")
                nc.gpsimd.indirect_dma_start(
                    out=m0[:],
                    out_offset=None,
                    in_=mrows[:, :],
                    in_offset=bass.IndirectOffsetOnAxis(
                        ap=gidx_sb[:, 2 * t : 2 * t + 1], axis=0
                    ),
                )
                nc.gpsimd.indirect_dma_start(
                    out=m1[:],
                    out_offset=None,
                    in_=mrows[:, :],
                    in_offset=bass.IndirectOffsetOnAxis(
                        ap=gidx_sb[:, 2 * t + 1 : 2 * t + 2], axis=0
                    ),
                )
                h0 = io_pool.tile([P, P], f32, tag="h")
                h1 = io_pool.tile([P, P], f32, tag="h")
                nc.sync.dma_start(out=h0[:], in_=headT[t, 0, :, :])
                nc.sync.dma_start(out=h1[:], in_=headT[t, 1, :, :])
                tl = io_pool.tile([P, DIM], f32, tag="tl")
                nc.sync.dma_start(out=tl[:], in_=tailt[t, :, :])

                ps = psum_pool.tile([P, DIM], f32)
                nc.tensor.matmul(out=ps[:], lhsT=h0[:], rhs=m0[:], start=True, stop=False)
                nc.tensor.matmul(out=ps[:], lhsT=h1[:], rhs=m1[:], start=False, stop=True)

                # (tensor_tensor_reduce would fuse these, but it faults the
                # HW in this environment — use two plain DVE ops instead)
                scratch = io_pool.tile([P, DIM], f32, tag="scratch")
                nc.vector.tensor_tensor(
                    out=scratch[:], in0=ps[:], in1=tl[:], op=mybir.AluOpType.mult
                )
                nc.vector.reduce_sum(
                    out=out_sb[:, t : t + 1],
                    in_=scratch[:],
                    axis=mybir.AxisListType.X,
                )

            nc.sync.dma_start(out=out[:, :], in_=out_sb[:])

    nc.compile()
    return nc


def _plan(ids, R):
    """Group sample indices by relation, chunk to <=128-row tiles, pad to a
    uniform per-core tile count. Returns (n_tiles, tiles) where tiles is a
    list of (relation, sample_indices) of length n_tiles * NCORES."""
    tiles = []
    for r in range(R):
        idxs = np.nonzero(ids == r)[0]
        for s in range(0, len(idxs), P):
            tiles.append((r, idxs[s : s + P]))
    if not tiles:
        tiles.append((0, np.empty(0, np.int64)))
    n_tiles = -(-len(tiles) // NCORES)
    empty = np.empty(0, np.int64)
    while len(tiles) < n_tiles * NCORES:
        tiles.append((0, empty))
    return n_tiles, tiles


def kernel(head, relation_ids, tail, relation_matrices):
    head = np.ascontiguousarray(np.asarray(head), dtype=np.float32)
    tail = np.ascontiguousarray(np.asarray(tail), dtype=np.float32)
    mats = np.ascontiguousarray(np.asarray(relation_matrices), dtype=np.float32)
    ids = np.asarray(relation_ids).astype(np.int64)
    B, D = head.shape
    R = mats.shape[0]
    assert D == DIM and R * DIM == 30 * DIM

    n_tiles, tiles = _plan(ids, R)
    mrows = mats.reshape(R * DIM, DIM)

    arange_p = np.arange(P, dtype=np.int32)
    in_maps = []
    for k in range(NCORES):
        ctiles = tiles[k * n_tiles : (k + 1) * n_tiles]
        headT = np.zeros((n_tiles, 2, P, P), np.float32)
        tailt = np.zeros((n_tiles, P, DIM), np.float32)
        gidxT = np.zeros((P, n_tiles * 2), np.int32)
        for t, (r, samp) in enumerate(ctiles):
            nb = len(samp)
            if nb:
                ht = head[samp].T  # [DIM, nb]
                headT[t, 0, :, :nb] = ht[:P]
                headT[t, 1, :, :nb] = ht[P:]
                tailt[t, :nb] = tail[samp]
            gidxT[:, 2 * t] = r * DIM + arange_p
            gidxT[:, 2 * t + 1] = r * DIM + P + arange_p
        in_maps.append(
            {"headT": headT, "tailt": tailt, "gidxT": gidxT, "mrows": mrows}
        )

    if n_tiles not in _prog_cache:
        _prog_cache[n_tiles] = _build(n_tiles)
    nc = _prog_cache[n_tiles]

    from concourse.bass_utils import run_bass_kernel_spmd

    kwargs = {}
    if TRACE:
        kwargs = dict(trace=True, trace_cores=list(range(NCORES)))
    res = run_bass_kernel_spmd(nc, in_maps, core_ids=list(range(NCORES)), **kwargs)
    global LAST_RESULT
    LAST_RESULT = res

    scores = np.zeros(B, np.float32)
    for k in range(NCORES):
        o = res.results[k]["out"]  # [P, n_tiles]
        ctiles = tiles[k * n_tiles : (k + 1) * n_tiles]
        for t, (r, samp) in enumerate(ctiles):
            nb = len(samp)
            if nb:
                scores[samp] = o[:nb, t]
    return scores
